# revision 16
# baseline (speedup 1.0000x reference)
"""Laplacian normalization kernel for Trainium2 (8 NeuronCores, SPMD).

out = D^-1/2 A D^-1/2 where D = diag(row sums of A), A: [8192, 8192] fp32.

Sharding: rows split across 8 cores (1024 rows each, 8 stripes of 128).

Single-read design, 64 MB/core HBM traffic (the floor):
  reads: eight fully-contiguous 4 MB SWDGE cast-DMAs (f32 HBM ->
    resident bf16 SBUF). Descriptors all pre-emitted -> gapless stream.
  row sums: DVE reduce over bf16 (+ reciprocal), ACT sqrt -> isq.
  collectives: a DUMMY warm-up AllGather fires at t=0 to absorb mesh
    init + launch skew off the critical path, then FOUR quarter
    AllGathers (one per stripe pair) so column scales unlock
    progressively while later stripes still stream.
  col-scale broadcast: gathered [2048] row -> tiny scalar-ring load ->
    TensorE outer product with a ones column into a PSUM tile that the
    DVE multiplies read directly as their column operand (no SBUF
    copy hop). Rowloads slot before each quarter's own stores, which
    wait on the same AG - a zero-cost FIFO position.
  pass B: out = (bf16A * r) * c via one fused scalar_tensor_tensor per
    (stripe, quarter) on DVE; quarter 0 emits before stripes 6-7's
    reduces (AG3 has mesh-chain slack to absorb the delay). Stores
    split across both HWDGE rings, never behind an AG-gated entry.

Error budget: bf16 A rounding ~2^-9 << the 2e-2 gate (col scales f32).
"""

import sys

sys.path.insert(0, "/opt/trn_rl_repo")

import numpy as np

import concourse.bacc as bacc
import concourse.tile as tile
from concourse import mybir
from concourse.bass_utils import run_bass_kernel_spmd

N = 8192          # full matrix dim
CORES = 8
R = N // CORES    # rows per core: 1024
P = 128           # partitions
S = R // P        # row stripes per core: 8
NQ = 4            # collective quarters (stripe pairs)
QAG = R // NQ     # isq elements per quarter: 256
W = CORES * QAG   # cb width per quarter: 2048
BK = 512          # PSUM bank width in f32
F32 = mybir.dt.float32
BF16 = mybir.dt.bfloat16
MUL = mybir.AluOpType.mult
X = mybir.AxisListType.X

_CACHE = {}


def build_nc():
    if "nc" in _CACHE:
        return _CACHE["nc"]
    nc = bacc.Bacc(
        "TRN2", target_bir_lowering=False, debug=False, num_devices=CORES
    )
    a = nc.dram_tensor("a_block", [R, N], F32, kind="ExternalInput").ap()
    out = nc.dram_tensor("out_block", [R, N], F32, kind="ExternalOutput").ap()

    with tile.TileContext(nc) as tc:
        with (
            tc.tile_pool(name="dram", bufs=1, space="DRAM") as dram,
            tc.tile_pool(name="res", bufs=1) as res,
            tc.tile_pool(name="work", bufs=4) as work,
            tc.tile_pool(name="cpool", bufs=1) as cpool,
            tc.tile_pool(name="small", bufs=1) as small,
            tc.tile_pool(name="psum", bufs=2, space="PSUM") as psum,
        ):
            isq_loc = [
                dram.tile([QAG], F32, name=f"isq_loc{q}") for q in range(NQ)
            ]
            isq_ag = [
                dram.tile(
                    [CORES * QAG], F32, addr_space="Shared", name=f"isq_ag{q}"
                )
                for q in range(NQ)
            ]
            warm_in = dram.tile([8], F32, name="warm_in")
            warm_out = dram.tile([64], F32, addr_space="Shared",
                                 name="warm_out")

            part = small.tile([P, S], F32)       # row sums -> 1/deg
            isq_sb = small.tile([P, S], F32)     # per-stripe row scale
            ones = small.tile([1, P], F32)       # outer-product column
            rowt = small.tile([1, W], F32, name="rowt")

            res_tiles = [
                res.tile([P, N], BF16, tag=f"res{s}", bufs=1, name=f"res{s}")
                for s in range(S)
            ]
            # col-scale tiles live in PSUM: cbp[q][p, m*QAG + u] = isq
            # of global row m*1024 + q*QAG + u = scale for that column,
            # written by the PE outer product and read by the DVE
            # multiplies directly (no SBUF copy hop)
            cbp = {}

            ag_args = dict(replica_groups=[list(range(CORES))])

            def q3(ap, q):
                """Quarter-q columns of [P, N] ap: within each 1024
                block, columns [q*QAG, (q+1)*QAG) -> [P, 8, QAG]."""
                return ap.rearrange("p (m c) -> p m c", c=R)[
                    :, :, q * QAG : (q + 1) * QAG
                ]

            stage = {}

            def mult(q, s, eng):
                o = work.tile([P, W], F32, tag="work")
                eng.scalar_tensor_tensor(
                    out=o[:].rearrange("p (m c) -> p m c", c=QAG),
                    in0=q3(res_tiles[s][:], q),
                    scalar=isq_sb[:, s : s + 1],
                    in1=cbp[q][:].rearrange("p (m c) -> p m c", c=QAG),
                    op0=MUL,
                    op1=MUL,
                )
                stage[(q, s)] = o

            def store(q, s, eng):
                eng.dma_start(
                    q3(out[s * P : (s + 1) * P, :], q),
                    stage.pop((q, s))[:].rearrange("p (m c) -> p m c", c=QAG),
                )

            def isqw(s):
                q, off = divmod(s * P, QAG)
                nc.sync.dma_start(
                    isq_loc[q][off : off + P].unsqueeze(1),
                    isq_sb[:, s : s + 1],
                )

            def rowload(q):
                # q0: scalar HWDGE ring (no stores queued yet, fires
                # at AG0 + low latency). q1-3: SWDGE, which is empty
                # after the reads drain and sits after the AG triggers
                # on the Pool queue — fires the instant AG_q lands,
                # never stuck behind another quarter's store drain.
                eng = nc.scalar if q == 0 else nc.gpsimd
                eng.dma_start(rowt[:], isq_ag[q][:].unsqueeze(0))

            def ag(q):
                nc.gpsimd.collective_compute(
                    "AllGather",
                    mybir.AluOpType.bypass,
                    ins=[isq_loc[q][:].opt()],
                    outs=[isq_ag[q][:].opt()],
                    **ag_args,
                )

            def bcast_mm(q):
                """PE outer product: psum[p, j] = ones[p] * rowt[q][j]."""
                pt = psum.tile([P, W], F32, tag="ps")
                for c in range(W // BK):
                    nc.tensor.matmul(
                        pt[:, c * BK : (c + 1) * BK],
                        ones[:],
                        rowt[:, c * BK : (c + 1) * BK],
                        start=True,
                        stop=True,
                    )
                cbp[q] = pt

            # ---- program (per-engine queues are in-order; the
            # emission order below is a hand-scheduled pipeline) ----

            # warm-up collective: absorbs mesh init + launch skew in
            # parallel with the read stream (input is garbage, output
            # unused, no deps)
            nc.gpsimd.collective_compute(
                "AllGather",
                mybir.AluOpType.bypass,
                ins=[warm_in[:].opt()],
                outs=[warm_out[:].opt()],
                **ag_args,
            )

            # all eight cast-reads up front on the SWDGE ring
            for s in range(S):
                nc.gpsimd.dma_start(res_tiles[s][:], a[s * P : (s + 1) * P, :])

            nc.vector.memset(ones[:], 1.0)

            # pass-A chain per stripe: DVE reduce+recip, ACT sqrt,
            # sync-ring isq write, AG trigger per stripe pair. Nothing
            # else touches these queues, so every core's isq chunks ship
            # at the earliest possible moment (the collectives are gated
            # by the slowest core's chunks, so its chain must stay tight)
            for s in range(S):
                nc.vector.reduce_sum(
                    out=part[:, s : s + 1], in_=res_tiles[s][:], axis=X
                )
                nc.vector.reciprocal(part[:, s : s + 1], part[:, s : s + 1])
                nc.scalar.sqrt(isq_sb[:, s : s + 1], part[:, s : s + 1])
                isqw(s)
                if s % 2 == 1 and s < 7:
                    ag(s // 2)
                # NOTE: emission order IS the dependency graph — each
                # rowload after the previous quarter's matmuls (rowt
                # WAR), each matmul group after the copy that drains
                # the PSUM banks it reuses (PSUM WAR).
                if s == 5:
                    # quarter-0 cb pipeline + early multiplies emit
                    # BEFORE stripes 6-7's reduces: they only need AG0,
                    # and AG3 has mesh-chain slack to absorb the
                    # delayed last reduces
                    rowload(0)
                    bcast_mm(0)
                    for t in range(6):
                        mult(0, t, nc.vector)
                        store(0, t, nc.scalar if t % 2 else nc.sync)

            mult(0, 6, nc.vector)
            store(0, 6, nc.sync)
            mult(0, 7, nc.vector)
            store(0, 7, nc.scalar)
            ag(3)

            # ---- pass B: quarters 1-3, stores balanced across both
            # HWDGE rings (pure stores, nothing AG-gated ahead of them)
            for q in range(1, NQ):
                rowload(q)
                bcast_mm(q)
                for s in range(S):
                    mult(q, s, nc.vector)
                    store(q, s, nc.scalar if s % 2 else nc.sync)

    nc.compile()
    _CACHE["nc"] = nc
    return nc


def kernel(adjacency_matrix):
    A = np.ascontiguousarray(np.asarray(adjacency_matrix, dtype=np.float32))
    assert A.shape == (N, N)
    nc = build_nc()
    in_maps = [
        {"a_block": np.ascontiguousarray(A[k * R : (k + 1) * R])}
        for k in range(CORES)
    ]
    res = run_bass_kernel_spmd(nc, in_maps, list(range(CORES)))
    return np.concatenate(
        [res.results[k]["out_block"] for k in range(CORES)], axis=0
    )
